# revision 19
# baseline (speedup 1.0000x reference)
"""Trainium2 Bass kernel for a single-step GRU decoder:
  x = relu(emb[input_ids]); GRU cell; logits = h_new @ W_out.T + b_out;
  out = log_softmax(logits).

Sharding across 8 NeuronCores:
  - gate dim (3H) sharded: each core computes 256 hidden units' worth of
    r/z/n gates and its h_new slice, then an AllGather of h_new (1KB).
  - vocab sharded: each core computes 6400 (padded) logits, local max /
    sum-exp, AllGather of the (max, sumexp) pairs, local normalize.

All matvecs run on the tensor engine with the activation column as the
*stationary* operand ([K=128, M=1], trivial weight load) and the weight
matrix as the *moving* operand ([K=128, N<=512]) — the big matrices
stream through the PE at line rate, so the kernel is DMA-bound.
Weights are pre-transposed on the host so every DMA is per-partition
contiguous.
"""

import os

import ml_dtypes
import numpy as np

import concourse.bass as bass
import concourse.bacc as bacc
import concourse.mybir as mybir
import concourse.tile as tile
from concourse.bass_utils import run_bass_kernel_spmd
from concourse.masks import make_identity

H = 2048
V = 50257
NC = 8
HC = H // NC          # 256 hidden units per core
KT = H // 128         # 16 contraction k-tiles
G3 = 3 * HC           # 768 gate rows per core (r/z/n x 256)
VSH = 6400            # padded vocab rows per core
VPAD = VSH * NC       # 51200
NFULL = 12            # full 512-wide logit chunks; last chunk is 256
CHUNKS = [(i * 512, 512) for i in range(NFULL)] + [(NFULL * 512, VSH - NFULL * 512)]

F32 = mybir.dt.float32
BF16 = mybir.dt.bfloat16
AF = mybir.ActivationFunctionType
AX = mybir.AxisListType


def build_program() -> bass.Bass:
    nc = bacc.Bacc(None, num_devices=NC)

    xrow = nc.declare_dram_parameter("xrow", [KT, 128], F32, isOutput=False)
    hrow = nc.declare_dram_parameter("hrow", [KT, 128], F32, isOutput=False)
    hsl = nc.declare_dram_parameter("hsl", [1, HC], F32, isOutput=False)
    wih = nc.declare_dram_parameter("wih", [2, 128, KT // 2, G3], F32, isOutput=False)
    whh = nc.declare_dram_parameter("whh", [2, 128, KT // 2, G3], F32, isOutput=False)
    bg = nc.declare_dram_parameter("bg", [1, 4 * HC], F32, isOutput=False)
    wout = nc.declare_dram_parameter(
        "wout", [NFULL, 128, KT, 512], BF16, isOutput=False
    )
    woutl = nc.declare_dram_parameter("woutl", [128, KT, CHUNKS[-1][1]], BF16, isOutput=False)
    bout = nc.declare_dram_parameter("bout", [1, VSH], F32, isOutput=False)
    logp = nc.declare_dram_parameter("logp", [1, VSH], F32, isOutput=True)
    hnew = nc.declare_dram_parameter("hnew", [1, HC], F32, isOutput=True)

    rg = [list(range(NC))]
    KH = KT // 2  # gate weights stream in two k-halves

    with tile.TileContext(nc) as tc:
        with (
            tc.tile_pool(name="singles", bufs=1) as singles,
            tc.tile_pool(name="gwpool", bufs=2) as gwpool,
            tc.tile_pool(name="wopool", bufs=4) as wopool,
            tc.tile_pool(name="small", bufs=2) as small,
            tc.tile_pool(name="tiny", bufs=1) as tiny,
            tc.tile_pool(name="ppg", bufs=1, space="PSUM") as ppg,
            tc.tile_pool(name="ppmm", bufs=3, space="PSUM") as ppmm,
            tc.tile_pool(name="ppmisc", bufs=2, space="PSUM") as ppmisc,
            tc.tile_pool(name="dram", bufs=1, space="DRAM") as drampool,
        ):
            ident = singles.tile([128, 128], F32)
            make_identity(nc, ident)

            # ---- x (emb row) and h as [128, KT] stationary-feed layouts ----
            xr_sb = small.tile([KT, 128], F32, tag="rowload")
            nc.scalar.dma_start(out=xr_sb, in_=xrow[:, :])
            xps = ppmisc.tile([128, KT], F32, tag="misc")
            nc.tensor.transpose(xps, xr_sb, ident[:KT, :KT])
            x_t = singles.tile([128, KT], F32)
            nc.scalar.activation(x_t, xps, AF.Relu)  # x = relu(emb row)

            hr_sb = small.tile([KT, 128], F32, tag="rowload")
            nc.scalar.dma_start(out=hr_sb, in_=hrow[:, :])
            hps = ppmisc.tile([128, KT], F32, tag="misc")
            nc.tensor.transpose(hps, hr_sb, ident[:KT, :KT])
            h_t = singles.tile([128, KT], F32)
            nc.vector.tensor_copy(h_t, hps)

            bg_sb = singles.tile([1, 4 * HC], F32)
            nc.scalar.dma_start(out=bg_sb, in_=bg[:, :])
            hsl_sb = singles.tile([1, HC], F32)
            nc.scalar.dma_start(out=hsl_sb, in_=hsl[:, :])

            # ---- gate weights: two k-halves per matrix, streamed ----
            gtiles = {}
            for name, src in (("ih", wih), ("hh", whh)):
                for half in range(2):
                    t = gwpool.tile([128, KH, G3], F32, tag="gw")
                    nc.sync.dma_start(out=t, in_=src[half])
                    gtiles[(name, half)] = t

            # ---- GRU gate matvecs (activation stationary, W moving) ----
            # Interleaved so each streamed weight tile is consumed fully in
            # DMA order (avoids pool-slot deadlock with 3 stream slots).
            psrz = ppg.tile([1, 512], F32, tag="rz")
            psnx = ppg.tile([1, HC], F32, tag="nx")
            psnh = ppg.tile([1, HC], F32, tag="nh")
            for k in range(KT):  # x side, consumes ih halves
                w = gtiles[("ih", k // KH)]
                nc.tensor.matmul(
                    psrz,
                    lhsT=x_t[:, k : k + 1],
                    rhs=w[:, k % KH, 0:512],
                    start=(k == 0),
                    stop=False,
                )
                nc.tensor.matmul(
                    psnx,
                    lhsT=x_t[:, k : k + 1],
                    rhs=w[:, k % KH, 512:768],
                    start=(k == 0),
                    stop=(k == KT - 1),
                )
            for k in range(KT):  # h side, consumes hh halves
                w = gtiles[("hh", k // KH)]
                nc.tensor.matmul(
                    psrz,
                    lhsT=h_t[:, k : k + 1],
                    rhs=w[:, k % KH, 0:512],
                    start=False,
                    stop=(k == KT - 1),
                )
                nc.tensor.matmul(
                    psnh,
                    lhsT=h_t[:, k : k + 1],
                    rhs=w[:, k % KH, 512:768],
                    start=(k == 0),
                    stop=(k == KT - 1),
                )

            # ---- gate nonlinearities (all on partition 0) ----
            rz_pre = small.tile([1, 512], F32, tag="g1")
            nc.vector.tensor_add(rz_pre, psrz, bg_sb[:, 0:512])
            rz = small.tile([1, 512], F32, tag="g2")
            nc.scalar.activation(rz, rz_pre, AF.Sigmoid)
            tnh = small.tile([1, HC], F32, tag="g3")  # nh + b_hh_n
            nc.vector.tensor_add(tnh, psnh, bg_sb[:, 3 * HC : 4 * HC])
            tr = small.tile([1, HC], F32, tag="g3")  # r * (nh + b_hh_n)
            nc.vector.tensor_mul(tr, rz[:, 0:HC], tnh)
            t2 = small.tile([1, HC], F32, tag="g3")
            nc.vector.tensor_add(t2, tr, psnx)
            t3 = small.tile([1, HC], F32, tag="g3")
            nc.vector.tensor_add(t3, t2, bg_sb[:, 2 * HC : 3 * HC])
            n_sb = small.tile([1, HC], F32, tag="g4")
            nc.scalar.activation(n_sb, t3, AF.Tanh)
            # h_new = n + z*(h - n)
            d_sb = small.tile([1, HC], F32, tag="g3")
            nc.vector.tensor_sub(d_sb, hsl_sb, n_sb)
            zd_sb = small.tile([1, HC], F32, tag="g3")
            nc.vector.tensor_mul(zd_sb, rz[:, HC : 2 * HC], d_sb)
            hn_sb = small.tile([1, HC], F32, tag="g4")
            nc.vector.tensor_add(hn_sb, n_sb, zd_sb)

            nc.scalar.dma_start(out=hnew[:, :], in_=hn_sb)

            # ---- AllGather h_new (256 floats per core -> 2048) ----
            ag_in = drampool.tile([1, HC], F32)
            ag_out = drampool.tile([KT, 128], F32)
            nc.scalar.dma_start(out=ag_in, in_=hn_sb)
            nc.gpsimd.collective_compute(
                "AllGather",
                mybir.AluOpType.bypass,
                replica_groups=rg,
                ins=[ag_in[:, :].opt()],
                outs=[ag_out[:, :].opt()],
            )
            agl = small.tile([KT, 128], F32, tag="rowload")
            nc.scalar.dma_start(out=agl, in_=ag_out[:, :])
            hxps = ppmisc.tile([128, KT], F32, tag="misc")
            nc.tensor.transpose(hxps, agl, ident[:KT, :KT])
            hx_t = singles.tile([128, KT], BF16)
            nc.vector.tensor_copy(hx_t, hxps)  # f32 psum -> bf16

            # ---- output projection: logits[1, 0:6400] ----
            logits = singles.tile([1, VSH], F32)
            bout_sb = singles.tile([1, VSH], F32)
            nc.scalar.dma_start(out=bout_sb, in_=bout[:, :])
            cmx = tiny.tile([1, len(CHUNKS)], F32, tag="cmx")
            for ci, (off, n) in enumerate(CHUNKS):
                wo_sb = wopool.tile([128, KT, n], BF16, tag="wo")
                src = wout[ci] if ci < NFULL else woutl[:, :, :]
                nc.sync.dma_start(out=wo_sb, in_=src)
                ps = ppmm.tile([1, n], F32, tag="mm")
                for k in range(KT):
                    nc.tensor.matmul(
                        ps,
                        lhsT=hx_t[:, k : k + 1],
                        rhs=wo_sb[:, k, :],
                        start=(k == 0),
                        stop=(k == KT - 1),
                    )
                nc.vector.tensor_add(
                    logits[:, off : off + n], ps, bout_sb[:, off : off + n]
                )
                # per-chunk max, overlapped with streaming
                nc.vector.tensor_reduce(
                    cmx[:, ci : ci + 1],
                    logits[:, off : off + n],
                    axis=AX.X,
                    op=mybir.AluOpType.max,
                )

            # ---- local log-softmax stats (single lane) ----
            mneg = tiny.tile([1, 1], F32, tag="mneg")  # -max(logits)
            nc.vector.tensor_reduce(
                mneg, cmx, axis=AX.X, op=mybir.AluOpType.max, negate=True
            )
            escr = singles.tile([1, VSH], BF16)
            sp = tiny.tile([1, 1], F32, tag="sp")  # sum exp(logits - max)
            nc.scalar.activation(escr, logits, AF.Exp, bias=mneg, accum_out=sp)

            stin_sb = tiny.tile([1, 2], F32, tag="stin")
            nc.vector.tensor_scalar_mul(stin_sb[:, 0:1], mneg, -1.0)  # m_c
            nc.vector.tensor_copy(stin_sb[:, 1:2], sp)

            # ---- AllGather (m_c, s_c) pairs ----
            st_in = drampool.tile([1, 2], F32)
            st_out = drampool.tile([NC, 2], F32)
            nc.scalar.dma_start(out=st_in, in_=stin_sb)
            nc.gpsimd.collective_compute(
                "AllGather",
                mybir.AluOpType.bypass,
                replica_groups=rg,
                ins=[st_in[:, :].opt()],
                outs=[st_out[:, :].opt()],
            )
            # strided loads: m values at even offsets, s at odd
            mg_row = small.tile([1, NC], F32, tag="sc6")
            nc.scalar.dma_start(out=mg_row, in_=st_out[:, 0:1].rearrange("r t -> t r"))
            sg_row = small.tile([1, NC], F32, tag="sc6")
            nc.scalar.dma_start(out=sg_row, in_=st_out[:, 1:2].rearrange("r t -> t r"))

            # global logsumexp: m = max_c m_c ; s = sum_c s_c * exp(m_c - m)
            mg_n = tiny.tile([1, 1], F32, tag="mg_n")  # -m
            nc.vector.tensor_reduce(
                mg_n, mg_row, axis=AX.X, op=mybir.AluOpType.max, negate=True
            )
            e2 = small.tile([1, NC], F32, tag="sc7")
            nc.scalar.activation(e2, mg_row, AF.Exp, bias=mg_n)
            t7 = small.tile([1, NC], F32, tag="sc7")
            nc.vector.tensor_mul(t7, e2, sg_row)
            sg = tiny.tile([1, 1], F32, tag="sg")
            nc.vector.tensor_reduce(sg, t7, axis=AX.X, op=mybir.AluOpType.add)
            lns = tiny.tile([1, 1], F32, tag="lns")
            nc.scalar.activation(lns, sg, AF.Ln)
            lse = tiny.tile([1, 1], F32, tag="lse")
            nc.vector.tensor_sub(lse, lns, mg_n)  # ln(s) + m

            nc.vector.tensor_scalar_sub(logits, logits, lse)  # in-place
            nc.scalar.dma_start(out=logp[:, :], in_=logits)

    nc.finalize()
    return nc


def _shard_inputs(input_ids, hidden, emb, W_ih, W_hh, b_ih, b_hh, W_out, b_out):
    f32 = lambda a: np.ascontiguousarray(np.asarray(a), dtype=np.float32)
    tok = int(np.asarray(input_ids).reshape(-1)[0])
    emb_row = f32(np.asarray(emb)[tok])          # [H]; relu happens on device
    h0 = f32(hidden).reshape(H)
    W_ih = f32(W_ih)
    W_hh = f32(W_hh)
    b_ih = f32(b_ih)
    b_hh = f32(b_hh)
    W_out = f32(W_out)
    b_out = f32(b_out)

    xrow = emb_row.reshape(KT, 128)
    hrow = h0.reshape(KT, 128)

    Wout_pad = np.zeros((VPAD, H), np.float32)
    Wout_pad[:V] = W_out
    bout_pad = np.full(VPAD, -1e30, np.float32)
    bout_pad[:V] = b_out

    bsum = b_ih + b_hh
    in_maps = []
    for c in range(NC):
        perm = np.concatenate([b * H + c * HC + np.arange(HC) for b in range(3)])
        # W[perm].T is [H, G3] with row k*128+kk; -> [half, kk, k_local, m]
        KH = KT // 2
        wih_c = np.ascontiguousarray(
            W_ih[perm].T.reshape(2, KH, 128, G3).transpose(0, 2, 1, 3)
        )
        whh_c = np.ascontiguousarray(
            W_hh[perm].T.reshape(2, KH, 128, G3).transpose(0, 2, 1, 3)
        )
        bgc = np.concatenate(
            [bsum[perm[: 2 * HC]], b_ih[perm[2 * HC :]], b_hh[perm[2 * HC :]]]
        )[None]
        hslc = h0[c * HC : (c + 1) * HC][None]

        WoT = (
            Wout_pad[c * VSH : (c + 1) * VSH]
            .T.astype(ml_dtypes.bfloat16)
            .reshape(KT, 128, VSH)
        )
        wout_c = np.ascontiguousarray(
            WoT[:, :, : NFULL * 512]
            .reshape(KT, 128, NFULL, 512)
            .transpose(2, 1, 0, 3)
        )
        woutl_c = np.ascontiguousarray(WoT[:, :, NFULL * 512 :].transpose(1, 0, 2))
        bout_c = bout_pad[c * VSH : (c + 1) * VSH][None]

        in_maps.append(
            {
                "xrow": xrow,
                "hrow": hrow,
                "hsl": np.ascontiguousarray(hslc),
                "wih": wih_c,
                "whh": whh_c,
                "bg": np.ascontiguousarray(bgc),
                "wout": wout_c,
                "woutl": woutl_c,
                "bout": np.ascontiguousarray(bout_c),
            }
        )
    return in_maps


def _unshard(results):
    logp = np.concatenate(
        [np.asarray(results[c]["logp"]).reshape(-1) for c in range(NC)]
    )[:V][None]
    hnew = np.concatenate(
        [np.asarray(results[c]["hnew"]).reshape(-1) for c in range(NC)]
    )[None, None]
    return logp.astype(np.float32), hnew.astype(np.float32)


_PROG = None
LAST_RUN = None  # BassKernelResults of the most recent kernel() call


def _get_prog():
    global _PROG
    if _PROG is None:
        _PROG = build_program()
    return _PROG


def _ensure_ntff_hook():
    """Register the axon NTFF profile hook if the image's antenv lacks it."""
    import sys
    import types

    try:
        from antenv.axon_hooks import get_axon_ntff_profile_hook  # noqa: F401

        return
    except ImportError:
        pass
    import antenv

    mod = types.ModuleType("antenv.axon_hooks")
    state = {"hook": None}
    mod.set_axon_ntff_profile_hook = lambda h: state.__setitem__("hook", h)
    mod.get_axon_ntff_profile_hook = lambda: state["hook"]
    sys.modules["antenv.axon_hooks"] = mod
    antenv.axon_hooks = mod
    try:
        from trn_agent_boot.trn_boot import _ntff_profile_via_ctypes

        mod.set_axon_ntff_profile_hook(
            _ntff_profile_via_ctypes("/opt/axon/libaxon_pjrt.so")
        )
    except Exception:
        pass  # hook stays None; bass_utils degrades to no-trace


def kernel(input_ids, hidden, emb, W_ih, W_hh, b_ih, b_hh, W_out, b_out):
    global LAST_RUN
    in_maps = _shard_inputs(
        input_ids, hidden, emb, W_ih, W_hh, b_ih, b_hh, W_out, b_out
    )
    nc = _get_prog()
    trace = bool(os.environ.get("BASS_KERNEL_TRACE"))
    if trace:
        _ensure_ntff_hook()
    LAST_RUN = run_bass_kernel_spmd(
        nc, in_maps, list(range(NC)), trace=trace
    )
    return _unshard(LAST_RUN.results)


# revision 20
# speedup vs baseline: 1.0297x; 1.0297x over previous
"""Trainium2 Bass kernel for a single-step GRU decoder:
  x = relu(emb[input_ids]); GRU cell; logits = h_new @ W_out.T + b_out;
  out = log_softmax(logits).

Sharding across 8 NeuronCores:
  - gate dim (3H) sharded: each core computes 256 hidden units' worth of
    r/z/n gates and its h_new slice, then an AllGather of h_new (1KB).
  - vocab sharded: each core computes 6400 (padded) logits, local max /
    sum-exp, AllGather of the (max, sumexp) pairs, local normalize.

All matvecs run on the tensor engine with the activation column as the
*stationary* operand ([K=128, M=1], trivial weight load) and the weight
matrix as the *moving* operand ([K=128, N<=512]) — the big matrices
stream through the PE at line rate, so the kernel is DMA-bound.
Weights are pre-transposed on the host so every DMA is per-partition
contiguous.
"""

import os

import ml_dtypes
import numpy as np

import concourse.bass as bass
import concourse.bacc as bacc
import concourse.mybir as mybir
import concourse.tile as tile
from concourse.bass_utils import run_bass_kernel_spmd
from concourse.masks import make_identity

H = 2048
V = 50257
NC = 8
HC = H // NC          # 256 hidden units per core
KT = H // 128         # 16 contraction k-tiles
G3 = 3 * HC           # 768 gate rows per core (r/z/n x 256)
VSH = 6400            # padded vocab rows per core
VPAD = VSH * NC       # 51200
NFULL = 12            # full 512-wide logit chunks; last chunk is 256
CHUNKS = [(i * 512, 512) for i in range(NFULL)] + [(NFULL * 512, VSH - NFULL * 512)]

F32 = mybir.dt.float32
BF16 = mybir.dt.bfloat16
AF = mybir.ActivationFunctionType
AX = mybir.AxisListType


def build_program() -> bass.Bass:
    nc = bacc.Bacc(None, num_devices=NC)

    xrow = nc.declare_dram_parameter("xrow", [KT, 128], F32, isOutput=False)
    hrow = nc.declare_dram_parameter("hrow", [KT, 128], F32, isOutput=False)
    hsl = nc.declare_dram_parameter("hsl", [1, HC], F32, isOutput=False)
    wih = nc.declare_dram_parameter("wih", [2, 128, KT // 2, G3], F32, isOutput=False)
    whh = nc.declare_dram_parameter("whh", [2, 128, KT // 2, G3], F32, isOutput=False)
    bg = nc.declare_dram_parameter("bg", [1, 4 * HC], F32, isOutput=False)
    wout = nc.declare_dram_parameter(
        "wout", [NFULL, 128, KT, 512], BF16, isOutput=False
    )
    woutl = nc.declare_dram_parameter("woutl", [128, KT, CHUNKS[-1][1]], BF16, isOutput=False)
    bout = nc.declare_dram_parameter("bout", [1, VSH], BF16, isOutput=False)
    logp = nc.declare_dram_parameter("logp", [1, VSH], F32, isOutput=True)
    hnew = nc.declare_dram_parameter("hnew", [1, HC], F32, isOutput=True)

    rg = [list(range(NC))]
    KH = KT // 2  # gate weights stream in two k-halves

    with tile.TileContext(nc) as tc:
        with (
            tc.tile_pool(name="singles", bufs=1) as singles,
            tc.tile_pool(name="gwpool", bufs=2) as gwpool,
            tc.tile_pool(name="wopool", bufs=5) as wopool,
            tc.tile_pool(name="small", bufs=2) as small,
            tc.tile_pool(name="tiny", bufs=1) as tiny,
            tc.tile_pool(name="ppg", bufs=1, space="PSUM") as ppg,
            tc.tile_pool(name="ppmm", bufs=3, space="PSUM") as ppmm,
            tc.tile_pool(name="ppmisc", bufs=2, space="PSUM") as ppmisc,
            tc.tile_pool(name="dram", bufs=1, space="DRAM") as drampool,
        ):
            ident = singles.tile([128, 128], F32)
            make_identity(nc, ident)

            # ---- x (emb row) and h as [128, KT] stationary-feed layouts ----
            xr_sb = small.tile([KT, 128], F32, tag="rowload")
            nc.scalar.dma_start(out=xr_sb, in_=xrow[:, :])
            xps = ppmisc.tile([128, KT], F32, tag="misc")
            nc.tensor.transpose(xps, xr_sb, ident[:KT, :KT])
            x_t = singles.tile([128, KT], F32)
            nc.scalar.activation(x_t, xps, AF.Relu)  # x = relu(emb row)

            hr_sb = small.tile([KT, 128], F32, tag="rowload")
            nc.scalar.dma_start(out=hr_sb, in_=hrow[:, :])
            hps = ppmisc.tile([128, KT], F32, tag="misc")
            nc.tensor.transpose(hps, hr_sb, ident[:KT, :KT])
            h_t = singles.tile([128, KT], F32)
            nc.vector.tensor_copy(h_t, hps)

            bg_sb = singles.tile([1, 4 * HC], F32)
            nc.scalar.dma_start(out=bg_sb, in_=bg[:, :])
            hsl_sb = singles.tile([1, HC], F32)
            nc.scalar.dma_start(out=hsl_sb, in_=hsl[:, :])

            # ---- gate weights: two k-halves per matrix, streamed ----
            gtiles = {}
            for name, src in (("ih", wih), ("hh", whh)):
                for half in range(2):
                    t = gwpool.tile([128, KH, G3], F32, tag="gw")
                    nc.sync.dma_start(out=t, in_=src[half])
                    gtiles[(name, half)] = t

            # ---- GRU gate matvecs (activation stationary, W moving) ----
            # Interleaved so each streamed weight tile is consumed fully in
            # DMA order (avoids pool-slot deadlock with 3 stream slots).
            psrz = ppg.tile([1, 512], F32, tag="rz")
            psnx = ppg.tile([1, HC], F32, tag="nx")
            psnh = ppg.tile([1, HC], F32, tag="nh")
            for k in range(KT):  # x side, consumes ih halves
                w = gtiles[("ih", k // KH)]
                nc.tensor.matmul(
                    psrz,
                    lhsT=x_t[:, k : k + 1],
                    rhs=w[:, k % KH, 0:512],
                    start=(k == 0),
                    stop=False,
                )
                nc.tensor.matmul(
                    psnx,
                    lhsT=x_t[:, k : k + 1],
                    rhs=w[:, k % KH, 512:768],
                    start=(k == 0),
                    stop=(k == KT - 1),
                )
            for k in range(KT):  # h side, consumes hh halves
                w = gtiles[("hh", k // KH)]
                nc.tensor.matmul(
                    psrz,
                    lhsT=h_t[:, k : k + 1],
                    rhs=w[:, k % KH, 0:512],
                    start=False,
                    stop=(k == KT - 1),
                )
                nc.tensor.matmul(
                    psnh,
                    lhsT=h_t[:, k : k + 1],
                    rhs=w[:, k % KH, 512:768],
                    start=(k == 0),
                    stop=(k == KT - 1),
                )

            # ---- gate nonlinearities (all on partition 0) ----
            rz_pre = small.tile([1, 512], F32, tag="g1")
            nc.vector.tensor_add(rz_pre, psrz, bg_sb[:, 0:512])
            rz = small.tile([1, 512], F32, tag="g2")
            nc.scalar.activation(rz, rz_pre, AF.Sigmoid)
            tnh = small.tile([1, HC], F32, tag="g3")  # nh + b_hh_n
            nc.vector.tensor_add(tnh, psnh, bg_sb[:, 3 * HC : 4 * HC])
            tr = small.tile([1, HC], F32, tag="g3")  # r * (nh + b_hh_n)
            nc.vector.tensor_mul(tr, rz[:, 0:HC], tnh)
            t2 = small.tile([1, HC], F32, tag="g3")
            nc.vector.tensor_add(t2, tr, psnx)
            t3 = small.tile([1, HC], F32, tag="g3")
            nc.vector.tensor_add(t3, t2, bg_sb[:, 2 * HC : 3 * HC])
            n_sb = small.tile([1, HC], F32, tag="g4")
            nc.scalar.activation(n_sb, t3, AF.Tanh)
            # h_new = n + z*(h - n)
            d_sb = small.tile([1, HC], F32, tag="g3")
            nc.vector.tensor_sub(d_sb, hsl_sb, n_sb)
            zd_sb = small.tile([1, HC], F32, tag="g3")
            nc.vector.tensor_mul(zd_sb, rz[:, HC : 2 * HC], d_sb)
            hn_sb = small.tile([1, HC], F32, tag="g4")
            nc.vector.tensor_add(hn_sb, n_sb, zd_sb)

            nc.scalar.dma_start(out=hnew[:, :], in_=hn_sb)

            # ---- AllGather h_new (256 floats per core -> 2048) ----
            ag_in = drampool.tile([1, HC], F32)
            ag_out = drampool.tile([KT, 128], F32)
            nc.scalar.dma_start(out=ag_in, in_=hn_sb)
            nc.gpsimd.collective_compute(
                "AllGather",
                mybir.AluOpType.bypass,
                replica_groups=rg,
                ins=[ag_in[:, :].opt()],
                outs=[ag_out[:, :].opt()],
            )
            agl = small.tile([KT, 128], F32, tag="rowload")
            nc.scalar.dma_start(out=agl, in_=ag_out[:, :])
            hxps = ppmisc.tile([128, KT], F32, tag="misc")
            nc.tensor.transpose(hxps, agl, ident[:KT, :KT])
            hx_t = singles.tile([128, KT], BF16)
            nc.vector.tensor_copy(hx_t, hxps)  # f32 psum -> bf16

            # ---- output projection: logits[1, 0:6400] ----
            logits = singles.tile([1, VSH], F32)
            bout_sb = singles.tile([1, VSH], BF16)
            nc.scalar.dma_start(out=bout_sb, in_=bout[:, :])
            cmx = tiny.tile([1, len(CHUNKS)], F32, tag="cmx")
            for ci, (off, n) in enumerate(CHUNKS):
                wo_sb = wopool.tile([128, KT, n], BF16, tag="wo")
                src = wout[ci] if ci < NFULL else woutl[:, :, :]
                nc.sync.dma_start(out=wo_sb, in_=src)
                ps = ppmm.tile([1, n], F32, tag="mm")
                for k in range(KT):
                    nc.tensor.matmul(
                        ps,
                        lhsT=hx_t[:, k : k + 1],
                        rhs=wo_sb[:, k, :],
                        start=(k == 0),
                        stop=(k == KT - 1),
                    )
                nc.vector.tensor_add(
                    logits[:, off : off + n], ps, bout_sb[:, off : off + n]
                )
                # per-chunk max, overlapped with streaming
                nc.vector.tensor_reduce(
                    cmx[:, ci : ci + 1],
                    logits[:, off : off + n],
                    axis=AX.X,
                    op=mybir.AluOpType.max,
                )

            # ---- local log-softmax stats (single lane) ----
            mneg = tiny.tile([1, 1], F32, tag="mneg")  # -max(logits)
            nc.vector.tensor_reduce(
                mneg, cmx, axis=AX.X, op=mybir.AluOpType.max, negate=True
            )
            escr = singles.tile([1, VSH], BF16)
            sp = tiny.tile([1, 1], F32, tag="sp")  # sum exp(logits - max)
            nc.scalar.activation(escr, logits, AF.Exp, bias=mneg, accum_out=sp)

            stin_sb = tiny.tile([1, 2], F32, tag="stin")
            nc.vector.tensor_scalar_mul(stin_sb[:, 0:1], mneg, -1.0)  # m_c
            nc.vector.tensor_copy(stin_sb[:, 1:2], sp)

            # ---- AllGather (m_c, s_c) pairs ----
            st_in = drampool.tile([1, 2], F32)
            st_out = drampool.tile([NC, 2], F32)
            nc.scalar.dma_start(out=st_in, in_=stin_sb)
            nc.gpsimd.collective_compute(
                "AllGather",
                mybir.AluOpType.bypass,
                replica_groups=rg,
                ins=[st_in[:, :].opt()],
                outs=[st_out[:, :].opt()],
            )
            # strided loads: m values at even offsets, s at odd
            mg_row = small.tile([1, NC], F32, tag="sc6")
            nc.scalar.dma_start(out=mg_row, in_=st_out[:, 0:1].rearrange("r t -> t r"))
            sg_row = small.tile([1, NC], F32, tag="sc6")
            nc.scalar.dma_start(out=sg_row, in_=st_out[:, 1:2].rearrange("r t -> t r"))

            # global logsumexp: m = max_c m_c ; s = sum_c s_c * exp(m_c - m)
            mg_n = tiny.tile([1, 1], F32, tag="mg_n")  # -m
            nc.vector.tensor_reduce(
                mg_n, mg_row, axis=AX.X, op=mybir.AluOpType.max, negate=True
            )
            e2 = small.tile([1, NC], F32, tag="sc7")
            nc.scalar.activation(e2, mg_row, AF.Exp, bias=mg_n)
            t7 = small.tile([1, NC], F32, tag="sc7")
            nc.vector.tensor_mul(t7, e2, sg_row)
            sg = tiny.tile([1, 1], F32, tag="sg")
            nc.vector.tensor_reduce(sg, t7, axis=AX.X, op=mybir.AluOpType.add)
            lns = tiny.tile([1, 1], F32, tag="lns")
            nc.scalar.activation(lns, sg, AF.Ln)
            lse = tiny.tile([1, 1], F32, tag="lse")
            nc.vector.tensor_sub(lse, lns, mg_n)  # ln(s) + m

            nc.vector.tensor_scalar_sub(logits, logits, lse)  # in-place
            nc.scalar.dma_start(out=logp[:, :], in_=logits)

    nc.finalize()
    return nc


def _shard_inputs(input_ids, hidden, emb, W_ih, W_hh, b_ih, b_hh, W_out, b_out):
    f32 = lambda a: np.ascontiguousarray(np.asarray(a), dtype=np.float32)
    tok = int(np.asarray(input_ids).reshape(-1)[0])
    emb_row = f32(np.asarray(emb)[tok])          # [H]; relu happens on device
    h0 = f32(hidden).reshape(H)
    W_ih = f32(W_ih)
    W_hh = f32(W_hh)
    b_ih = f32(b_ih)
    b_hh = f32(b_hh)
    W_out = f32(W_out)
    b_out = f32(b_out)

    xrow = emb_row.reshape(KT, 128)
    hrow = h0.reshape(KT, 128)

    Wout_pad = np.zeros((VPAD, H), np.float32)
    Wout_pad[:V] = W_out
    bout_pad = np.full(VPAD, -1e30, np.float32)
    bout_pad[:V] = b_out

    bsum = b_ih + b_hh
    in_maps = []
    for c in range(NC):
        perm = np.concatenate([b * H + c * HC + np.arange(HC) for b in range(3)])
        # W[perm].T is [H, G3] with row k*128+kk; -> [half, kk, k_local, m]
        KH = KT // 2
        wih_c = np.ascontiguousarray(
            W_ih[perm].T.reshape(2, KH, 128, G3).transpose(0, 2, 1, 3)
        )
        whh_c = np.ascontiguousarray(
            W_hh[perm].T.reshape(2, KH, 128, G3).transpose(0, 2, 1, 3)
        )
        bgc = np.concatenate(
            [bsum[perm[: 2 * HC]], b_ih[perm[2 * HC :]], b_hh[perm[2 * HC :]]]
        )[None]
        hslc = h0[c * HC : (c + 1) * HC][None]

        WoT = (
            Wout_pad[c * VSH : (c + 1) * VSH]
            .T.astype(ml_dtypes.bfloat16)
            .reshape(KT, 128, VSH)
        )
        wout_c = np.ascontiguousarray(
            WoT[:, :, : NFULL * 512]
            .reshape(KT, 128, NFULL, 512)
            .transpose(2, 1, 0, 3)
        )
        woutl_c = np.ascontiguousarray(WoT[:, :, NFULL * 512 :].transpose(1, 0, 2))
        bout_c = bout_pad[c * VSH : (c + 1) * VSH][None].astype(ml_dtypes.bfloat16)

        in_maps.append(
            {
                "xrow": xrow,
                "hrow": hrow,
                "hsl": np.ascontiguousarray(hslc),
                "wih": wih_c,
                "whh": whh_c,
                "bg": np.ascontiguousarray(bgc),
                "wout": wout_c,
                "woutl": woutl_c,
                "bout": np.ascontiguousarray(bout_c),
            }
        )
    return in_maps


def _unshard(results):
    logp = np.concatenate(
        [np.asarray(results[c]["logp"]).reshape(-1) for c in range(NC)]
    )[:V][None]
    hnew = np.concatenate(
        [np.asarray(results[c]["hnew"]).reshape(-1) for c in range(NC)]
    )[None, None]
    return logp.astype(np.float32), hnew.astype(np.float32)


_PROG = None
LAST_RUN = None  # BassKernelResults of the most recent kernel() call


def _get_prog():
    global _PROG
    if _PROG is None:
        _PROG = build_program()
    return _PROG


def _ensure_ntff_hook():
    """Register the axon NTFF profile hook if the image's antenv lacks it."""
    import sys
    import types

    try:
        from antenv.axon_hooks import get_axon_ntff_profile_hook  # noqa: F401

        return
    except ImportError:
        pass
    import antenv

    mod = types.ModuleType("antenv.axon_hooks")
    state = {"hook": None}
    mod.set_axon_ntff_profile_hook = lambda h: state.__setitem__("hook", h)
    mod.get_axon_ntff_profile_hook = lambda: state["hook"]
    sys.modules["antenv.axon_hooks"] = mod
    antenv.axon_hooks = mod
    try:
        from trn_agent_boot.trn_boot import _ntff_profile_via_ctypes

        mod.set_axon_ntff_profile_hook(
            _ntff_profile_via_ctypes("/opt/axon/libaxon_pjrt.so")
        )
    except Exception:
        pass  # hook stays None; bass_utils degrades to no-trace


def kernel(input_ids, hidden, emb, W_ih, W_hh, b_ih, b_hh, W_out, b_out):
    global LAST_RUN
    in_maps = _shard_inputs(
        input_ids, hidden, emb, W_ih, W_hh, b_ih, b_hh, W_out, b_out
    )
    nc = _get_prog()
    trace = bool(os.environ.get("BASS_KERNEL_TRACE"))
    if trace:
        _ensure_ntff_hook()
    LAST_RUN = run_bass_kernel_spmd(
        nc, in_maps, list(range(NC)), trace=trace
    )
    return _unshard(LAST_RUN.results)


# revision 23
# speedup vs baseline: 1.1401x; 1.1072x over previous
"""Trainium2 Bass kernel for a single-step GRU decoder:
  x = relu(emb[input_ids]); GRU cell; logits = h_new @ W_out.T + b_out;
  out = log_softmax(logits).

Sharding across 8 NeuronCores:
  - gate dim (3H) sharded: each core computes 256 hidden units' worth of
    r/z/n gates and its h_new slice, then an AllGather of h_new (1KB).
  - vocab sharded: each core computes 6400 (padded) logits, local max /
    sum-exp, AllGather of the (max, sumexp) pairs, local normalize.

All matvecs run on the tensor engine with the activation column as the
*stationary* operand ([K=128, M=1], trivial weight load) and the weight
matrix as the *moving* operand ([K=128, N<=512]) — the big matrices
stream through the PE at line rate, so the kernel is DMA-bound.
Weights are pre-transposed on the host so every DMA is per-partition
contiguous.
"""

import os

import ml_dtypes
import numpy as np

import concourse.bass as bass
import concourse.bacc as bacc
import concourse.mybir as mybir
import concourse.tile as tile
from concourse.bass_utils import run_bass_kernel_spmd
from concourse.masks import make_identity

H = 2048
V = 50257
NC = 8
HC = H // NC          # 256 hidden units per core
KT = H // 128         # 16 contraction k-tiles
G3 = 3 * HC           # 768 gate rows per core (r/z/n x 256)
VSH = 6400            # padded vocab rows per core
VPAD = VSH * NC       # 51200
NFULL = 12            # full 512-wide logit chunks; last chunk is 256
CHUNKS = [(i * 512, 512) for i in range(NFULL)] + [(NFULL * 512, VSH - NFULL * 512)]

F32 = mybir.dt.float32
BF16 = mybir.dt.bfloat16
AF = mybir.ActivationFunctionType
AX = mybir.AxisListType


def build_program() -> bass.Bass:
    nc = bacc.Bacc(None, num_devices=NC)

    xrow = nc.declare_dram_parameter("xrow", [KT, 128], F32, isOutput=False)
    hrow = nc.declare_dram_parameter("hrow", [KT, 128], F32, isOutput=False)
    hsl = nc.declare_dram_parameter("hsl", [1, HC], F32, isOutput=False)
    wih = nc.declare_dram_parameter("wih", [2, 128, KT // 2, G3], BF16, isOutput=False)
    whh = nc.declare_dram_parameter("whh", [2, 128, KT // 2, G3], BF16, isOutput=False)
    bg = nc.declare_dram_parameter("bg", [1, 4 * HC], F32, isOutput=False)
    wout = nc.declare_dram_parameter(
        "wout", [NFULL, 128, KT, 512], BF16, isOutput=False
    )
    woutl = nc.declare_dram_parameter("woutl", [128, KT, CHUNKS[-1][1]], BF16, isOutput=False)
    bout = nc.declare_dram_parameter("bout", [1, VSH], BF16, isOutput=False)
    logp = nc.declare_dram_parameter("logp", [1, VSH], F32, isOutput=True)
    hnew = nc.declare_dram_parameter("hnew", [1, HC], F32, isOutput=True)

    rg = [list(range(NC))]
    KH = KT // 2  # gate weights stream in two k-halves

    with tile.TileContext(nc) as tc:
        with (
            tc.tile_pool(name="singles", bufs=1) as singles,
            tc.tile_pool(name="gwpool", bufs=2) as gwpool,
            tc.tile_pool(name="wopool", bufs=6) as wopool,
            tc.tile_pool(name="small", bufs=2) as small,
            tc.tile_pool(name="tiny", bufs=1) as tiny,
            tc.tile_pool(name="ppg", bufs=1, space="PSUM") as ppg,
            tc.tile_pool(name="ppmm", bufs=3, space="PSUM") as ppmm,
            tc.tile_pool(name="ppmisc", bufs=2, space="PSUM") as ppmisc,
            tc.tile_pool(name="dram", bufs=1, space="DRAM") as drampool,
        ):
            ident = singles.tile([128, 128], F32)
            make_identity(nc, ident)

            # ---- x (emb row) and h as [128, KT] stationary-feed layouts ----
            xr_sb = small.tile([KT, 128], F32, tag="rowload")
            nc.scalar.dma_start(out=xr_sb, in_=xrow[:, :])
            xps = ppmisc.tile([128, KT], F32, tag="misc")
            nc.tensor.transpose(xps, xr_sb, ident[:KT, :KT])
            x_t = singles.tile([128, KT], BF16)
            nc.vector.tensor_scalar_max(x_t, xps, 0.0)  # x = relu(emb row), cast bf16

            hr_sb = small.tile([KT, 128], F32, tag="rowload")
            nc.scalar.dma_start(out=hr_sb, in_=hrow[:, :])
            hps = ppmisc.tile([128, KT], F32, tag="misc")
            nc.tensor.transpose(hps, hr_sb, ident[:KT, :KT])
            h_t = singles.tile([128, KT], BF16)
            nc.vector.tensor_copy(h_t, hps)

            bg_sb = singles.tile([1, 4 * HC], F32)
            nc.scalar.dma_start(out=bg_sb, in_=bg[:, :])
            hsl_sb = singles.tile([1, HC], F32)
            nc.scalar.dma_start(out=hsl_sb, in_=hsl[:, :])

            # ---- gate weights: two k-halves per matrix, streamed ----
            gtiles = {}
            for name, src in (("ih", wih), ("hh", whh)):
                for half in range(2):
                    t = gwpool.tile([128, KH, G3], BF16, tag="gw")
                    nc.sync.dma_start(out=t, in_=src[half])
                    gtiles[(name, half)] = t

            # ---- GRU gate matvecs (activation stationary, W moving) ----
            # Interleaved so each streamed weight tile is consumed fully in
            # DMA order (avoids pool-slot deadlock with 3 stream slots).
            psrz = ppg.tile([1, 512], F32, tag="rz")
            psnx = ppg.tile([1, HC], F32, tag="nx")
            psnh = ppg.tile([1, HC], F32, tag="nh")
            for k in range(KT):  # x side, consumes ih halves
                w = gtiles[("ih", k // KH)]
                nc.tensor.matmul(
                    psrz,
                    lhsT=x_t[:, k : k + 1],
                    rhs=w[:, k % KH, 0:512],
                    start=(k == 0),
                    stop=False,
                )
                nc.tensor.matmul(
                    psnx,
                    lhsT=x_t[:, k : k + 1],
                    rhs=w[:, k % KH, 512:768],
                    start=(k == 0),
                    stop=(k == KT - 1),
                )
            for k in range(KT):  # h side, consumes hh halves
                w = gtiles[("hh", k // KH)]
                nc.tensor.matmul(
                    psrz,
                    lhsT=h_t[:, k : k + 1],
                    rhs=w[:, k % KH, 0:512],
                    start=False,
                    stop=(k == KT - 1),
                )
                nc.tensor.matmul(
                    psnh,
                    lhsT=h_t[:, k : k + 1],
                    rhs=w[:, k % KH, 512:768],
                    start=(k == 0),
                    stop=(k == KT - 1),
                )

            # ---- gate nonlinearities (all on partition 0) ----
            rz_pre = small.tile([1, 512], F32, tag="g1")
            nc.vector.tensor_add(rz_pre, psrz, bg_sb[:, 0:512])
            rz = small.tile([1, 512], F32, tag="g2")
            nc.scalar.activation(rz, rz_pre, AF.Sigmoid)
            tnh = small.tile([1, HC], F32, tag="g3")  # nh + b_hh_n
            nc.vector.tensor_add(tnh, psnh, bg_sb[:, 3 * HC : 4 * HC])
            tr = small.tile([1, HC], F32, tag="g3")  # r * (nh + b_hh_n)
            nc.vector.tensor_mul(tr, rz[:, 0:HC], tnh)
            t2 = small.tile([1, HC], F32, tag="g3")
            nc.vector.tensor_add(t2, tr, psnx)
            t3 = small.tile([1, HC], F32, tag="g3")
            nc.vector.tensor_add(t3, t2, bg_sb[:, 2 * HC : 3 * HC])
            n_sb = small.tile([1, HC], F32, tag="g4")
            nc.scalar.activation(n_sb, t3, AF.Tanh)
            # h_new = n + z*(h - n)
            d_sb = small.tile([1, HC], F32, tag="g3")
            nc.vector.tensor_sub(d_sb, hsl_sb, n_sb)
            zd_sb = small.tile([1, HC], F32, tag="g3")
            nc.vector.tensor_mul(zd_sb, rz[:, HC : 2 * HC], d_sb)
            hn_sb = small.tile([1, HC], F32, tag="g4")
            nc.vector.tensor_add(hn_sb, n_sb, zd_sb)

            nc.scalar.dma_start(out=hnew[:, :], in_=hn_sb)
            gwarm = tiny.tile([1, 8], F32, tag="gwarm")
            nc.gpsimd.tensor_copy(gwarm, rz[:, 0:8])      # wake Q7 early
            nc.gpsimd.tensor_copy(gwarm, hn_sb[:, 0:8])

            # ---- AllGather h_new (256 floats per core -> 2048) ----
            ag_in = drampool.tile([1, HC], F32)
            ag_out = drampool.tile([KT, 128], F32)
            nc.scalar.dma_start(out=ag_in, in_=hn_sb)
            nc.gpsimd.collective_compute(
                "AllGather",
                mybir.AluOpType.bypass,
                replica_groups=rg,
                ins=[ag_in[:, :].opt()],
                outs=[ag_out[:, :].opt()],
            )
            agl = small.tile([KT, 128], F32, tag="rowload")
            nc.scalar.dma_start(out=agl, in_=ag_out[:, :])
            hxps = ppmisc.tile([128, KT], F32, tag="misc")
            nc.tensor.transpose(hxps, agl, ident[:KT, :KT])
            hx_t = singles.tile([128, KT], BF16)
            nc.vector.tensor_copy(hx_t, hxps)  # f32 psum -> bf16

            # ---- output projection: logits[1, 0:6400] ----
            logits = singles.tile([1, VSH], F32)
            bout_sb = singles.tile([1, VSH], BF16)
            nc.scalar.dma_start(out=bout_sb, in_=bout[:, :])
            cmx = tiny.tile([1, len(CHUNKS)], F32, tag="cmx")
            for ci, (off, n) in enumerate(CHUNKS):
                wo_sb = wopool.tile([128, KT, n], BF16, tag="wo")
                src = wout[ci] if ci < NFULL else woutl[:, :, :]
                nc.sync.dma_start(out=wo_sb, in_=src)
                ps = ppmm.tile([1, n], F32, tag="mm")
                for k in range(KT):
                    nc.tensor.matmul(
                        ps,
                        lhsT=hx_t[:, k : k + 1],
                        rhs=wo_sb[:, k, :],
                        start=(k == 0),
                        stop=(k == KT - 1),
                    )
                nc.vector.tensor_add(
                    logits[:, off : off + n], ps, bout_sb[:, off : off + n]
                )
                # per-chunk max, overlapped with streaming
                nc.vector.tensor_reduce(
                    cmx[:, ci : ci + 1],
                    logits[:, off : off + n],
                    axis=AX.X,
                    op=mybir.AluOpType.max,
                )

            # ---- local log-softmax stats (single lane) ----
            mneg = tiny.tile([1, 1], F32, tag="mneg")  # -max(logits)
            nc.vector.tensor_reduce(
                mneg, cmx, axis=AX.X, op=mybir.AluOpType.max, negate=True
            )
            escr = singles.tile([1, VSH], BF16)
            sp = tiny.tile([1, 1], F32, tag="sp")  # sum exp(logits - max)
            nc.scalar.activation(escr, logits, AF.Exp, bias=mneg, accum_out=sp)

            stin_sb = tiny.tile([1, 2], F32, tag="stin")
            nc.vector.tensor_scalar_mul(stin_sb[:, 0:1], mneg, -1.0)  # m_c
            nc.vector.tensor_copy(stin_sb[:, 1:2], sp)

            # ---- AllGather (m_c, s_c) pairs ----
            st_in = drampool.tile([1, 2], F32)
            st_out = drampool.tile([NC, 2], F32)
            nc.gpsimd.tensor_copy(gwarm, logits[:, 0:8])  # wake Q7 early
            nc.gpsimd.tensor_copy(gwarm[:, 0:2], stin_sb)
            nc.scalar.dma_start(out=st_in, in_=stin_sb)
            nc.gpsimd.collective_compute(
                "AllGather",
                mybir.AluOpType.bypass,
                replica_groups=rg,
                ins=[st_in[:, :].opt()],
                outs=[st_out[:, :].opt()],
            )
            # strided loads: m values at even offsets, s at odd
            mg_row = small.tile([1, NC], F32, tag="sc6")
            nc.scalar.dma_start(out=mg_row, in_=st_out[:, 0:1].rearrange("r t -> t r"))
            sg_row = small.tile([1, NC], F32, tag="sc6")
            nc.scalar.dma_start(out=sg_row, in_=st_out[:, 1:2].rearrange("r t -> t r"))

            # global logsumexp: m = max_c m_c ; s = sum_c s_c * exp(m_c - m)
            mg_n = tiny.tile([1, 1], F32, tag="mg_n")  # -m
            nc.vector.tensor_reduce(
                mg_n, mg_row, axis=AX.X, op=mybir.AluOpType.max, negate=True
            )
            e2 = small.tile([1, NC], F32, tag="sc7")
            nc.scalar.activation(e2, mg_row, AF.Exp, bias=mg_n)
            t7 = small.tile([1, NC], F32, tag="sc7")
            nc.vector.tensor_mul(t7, e2, sg_row)
            sg = tiny.tile([1, 1], F32, tag="sg")
            nc.vector.tensor_reduce(sg, t7, axis=AX.X, op=mybir.AluOpType.add)
            lns = tiny.tile([1, 1], F32, tag="lns")
            nc.scalar.activation(lns, sg, AF.Ln)
            lse = tiny.tile([1, 1], F32, tag="lse")
            nc.vector.tensor_sub(lse, lns, mg_n)  # ln(s) + m

            for qi in range(4):
                q0, q1 = qi * (VSH // 4), (qi + 1) * (VSH // 4)
                nc.vector.tensor_scalar_sub(
                    logits[:, q0:q1], logits[:, q0:q1], lse
                )  # in-place
                nc.scalar.dma_start(out=logp[:, q0:q1], in_=logits[:, q0:q1])

    nc.finalize()
    return nc


def _shard_inputs(input_ids, hidden, emb, W_ih, W_hh, b_ih, b_hh, W_out, b_out):
    f32 = lambda a: np.ascontiguousarray(np.asarray(a), dtype=np.float32)
    tok = int(np.asarray(input_ids).reshape(-1)[0])
    emb_row = f32(np.asarray(emb)[tok])          # [H]; relu happens on device
    h0 = f32(hidden).reshape(H)
    W_ih = f32(W_ih)
    W_hh = f32(W_hh)
    b_ih = f32(b_ih)
    b_hh = f32(b_hh)
    W_out = f32(W_out)
    b_out = f32(b_out)

    xrow = emb_row.reshape(KT, 128)
    hrow = h0.reshape(KT, 128)

    Wout_pad = np.zeros((VPAD, H), np.float32)
    Wout_pad[:V] = W_out
    bout_pad = np.full(VPAD, -1e30, np.float32)
    bout_pad[:V] = b_out

    bsum = b_ih + b_hh
    in_maps = []
    for c in range(NC):
        perm = np.concatenate([b * H + c * HC + np.arange(HC) for b in range(3)])
        # W[perm].T is [H, G3] with row k*128+kk; -> [half, kk, k_local, m]
        KH = KT // 2
        wih_c = np.ascontiguousarray(
            W_ih[perm].T.astype(ml_dtypes.bfloat16)
            .reshape(2, KH, 128, G3)
            .transpose(0, 2, 1, 3)
        )
        whh_c = np.ascontiguousarray(
            W_hh[perm].T.astype(ml_dtypes.bfloat16)
            .reshape(2, KH, 128, G3)
            .transpose(0, 2, 1, 3)
        )
        bgc = np.concatenate(
            [bsum[perm[: 2 * HC]], b_ih[perm[2 * HC :]], b_hh[perm[2 * HC :]]]
        )[None]
        hslc = h0[c * HC : (c + 1) * HC][None]

        WoT = (
            Wout_pad[c * VSH : (c + 1) * VSH]
            .T.astype(ml_dtypes.bfloat16)
            .reshape(KT, 128, VSH)
        )
        wout_c = np.ascontiguousarray(
            WoT[:, :, : NFULL * 512]
            .reshape(KT, 128, NFULL, 512)
            .transpose(2, 1, 0, 3)
        )
        woutl_c = np.ascontiguousarray(WoT[:, :, NFULL * 512 :].transpose(1, 0, 2))
        bout_c = bout_pad[c * VSH : (c + 1) * VSH][None].astype(ml_dtypes.bfloat16)

        in_maps.append(
            {
                "xrow": xrow,
                "hrow": hrow,
                "hsl": np.ascontiguousarray(hslc),
                "wih": wih_c,
                "whh": whh_c,
                "bg": np.ascontiguousarray(bgc),
                "wout": wout_c,
                "woutl": woutl_c,
                "bout": np.ascontiguousarray(bout_c),
            }
        )
    return in_maps


def _unshard(results):
    logp = np.concatenate(
        [np.asarray(results[c]["logp"]).reshape(-1) for c in range(NC)]
    )[:V][None]
    hnew = np.concatenate(
        [np.asarray(results[c]["hnew"]).reshape(-1) for c in range(NC)]
    )[None, None]
    return logp.astype(np.float32), hnew.astype(np.float32)


_PROG = None
LAST_RUN = None  # BassKernelResults of the most recent kernel() call


def _get_prog():
    global _PROG
    if _PROG is None:
        _PROG = build_program()
    return _PROG


def _ensure_ntff_hook():
    """Register the axon NTFF profile hook if the image's antenv lacks it."""
    import sys
    import types

    try:
        from antenv.axon_hooks import get_axon_ntff_profile_hook  # noqa: F401

        return
    except ImportError:
        pass
    import antenv

    mod = types.ModuleType("antenv.axon_hooks")
    state = {"hook": None}
    mod.set_axon_ntff_profile_hook = lambda h: state.__setitem__("hook", h)
    mod.get_axon_ntff_profile_hook = lambda: state["hook"]
    sys.modules["antenv.axon_hooks"] = mod
    antenv.axon_hooks = mod
    try:
        from trn_agent_boot.trn_boot import _ntff_profile_via_ctypes

        mod.set_axon_ntff_profile_hook(
            _ntff_profile_via_ctypes("/opt/axon/libaxon_pjrt.so")
        )
    except Exception:
        pass  # hook stays None; bass_utils degrades to no-trace


def kernel(input_ids, hidden, emb, W_ih, W_hh, b_ih, b_hh, W_out, b_out):
    global LAST_RUN
    in_maps = _shard_inputs(
        input_ids, hidden, emb, W_ih, W_hh, b_ih, b_hh, W_out, b_out
    )
    nc = _get_prog()
    trace = bool(os.environ.get("BASS_KERNEL_TRACE"))
    if trace:
        _ensure_ntff_hook()
    LAST_RUN = run_bass_kernel_spmd(
        nc, in_maps, list(range(NC)), trace=trace
    )
    return _unshard(LAST_RUN.results)
